# revision 8
# baseline (speedup 1.0000x reference)
"""Trainium2 Bass kernel for nn_Decoder_9586367004779 (social-GAN style decoder).

Self-contained: takes FULL inputs (as produced by the problem's setup_inputs()),
shards 16 scenes x 32 peds across 8 NeuronCores (2 scenes/core, fully
data-parallel, no collectives), runs a Bass/Tile kernel per core, and
reassembles the full [T, 512, 2] output.

Algorithm notes (vs the jax reference):
  * y_hard + y_soft - stop_gradient(y_soft) == y_hard numerically, so the
    gumbel pooling reduces to "raw at argmax_j(log(raw+eps)+g)". log is
    monotone, so argmax_j((raw+eps)*exp(g)) selects the same j; we drop the
    +eps (only reorders degenerate near-ties) and precompute
    eg = exp(g) = 1/(GEPS - log(u+GEPS)) on host with jax CPU threefry --
    bit-identical to the reference's random stream.  On device:
    score = relu(pre)*eg ; m = grouped max over j ; pooled = sum_j
    relu(pre)*[score==m].
  * Everything on device lives transposed [feature, batch] so all matmuls
    chain on the PE without transposes.  The pairwise-pool input
    concat([rel@W_pse, h_j]) @ W_p1 is folded to
    P_j@(W_pse@W_p1[:E]) - P_i@(W_pse@W_p1[:E]) + h_j@W_p1[E:] + b_eff
    using broadcast access patterns as matmul rhs (no materialized rel/hj).
"""

import functools
import numpy as np

import concourse.bass as bass
import concourse.mybir as mybir
import concourse.tile as tile
from concourse import bacc
from concourse.bass_utils import run_bass_kernel_spmd

S, P, H, E, BN, MH, T = 16, 32, 128, 64, 1024, 1024, 8
B = S * P
NCORES = 8
SLOC = S // NCORES          # scenes per core
BLOC = SLOC * P             # peds per core (64)
R = SLOC * P * P            # pairs per core (2048)
NT = R // 512               # 512-wide N tiles per pair-range (4)
DC = BN // 128              # d-chunks (8)
KC1 = 512 // 128            # k chunks of x1 (4)
GEPS, LEPS = 1e-10, 1e-8
FP = mybir.dt.float32
AX = mybir.AxisListType
AF = mybir.ActivationFunctionType
ALU = mybir.AluOpType


def _ap(ap_obj, dims):
    """Rebuild an AP with explicit [step, count] free dims (partition dim kept)."""
    base = list(map(list, ap_obj.ap))
    return bass.AP(tensor=ap_obj.tensor, offset=ap_obj.offset, ap=[base[0]] + dims)


def _bview(tile2d, col_off, dims):
    """View of a [P, N] tile starting at free offset col_off with free dims
    given as [step, count] pairs (step 0 = broadcast)."""
    sl = tile2d[:, col_off:]
    base = list(map(list, sl.ap))
    return bass.AP(tensor=sl.tensor, offset=sl.offset, ap=[base[0]] + dims)


def build_nc(t_steps=T, sel_engines=None):
    """Build the SPMD Bass program for one core."""
    nc = bacc.Bacc("TRN2", target_bir_lowering=False, debug=False)
    sel = sel_engines or {}
    v = nc.vector
    eng_score = getattr(nc, sel.get("score", "vector"))
    eng_eq = getattr(nc, sel.get("eq", "vector"))
    eng_p1 = getattr(nc, sel.get("p1", "vector"))

    # ---- dram parameters ----
    din = lambda name, shape: nc.declare_dram_parameter(name, shape, FP, isOutput=False)
    lr0 = din("lr0", [2, BLOC])          # last_pos_rel^T
    lp0 = din("lp0", [2, BLOC])          # last_pos^T
    h0 = din("h0t", [H, BLOC])
    c0 = din("c0t", [H, BLOC])
    w_se = din("w_se", [2, E])
    w_ih = din("w_ih", [E, 4 * H])
    w_hh = din("w_hh", [H, 4 * H])
    b_gates = din("b_gates", [H, 4])     # b_se@W_ih + b_ih + b_hh, col-chunked
    w_hp = din("w_hp", [H, 2])
    b_hp = din("b_hp", [2, 1])
    w_eff1 = din("w_eff1", [2, 512])     # W_pse @ W_p1[:E]
    nw_eff1 = din("nw_eff1", [2, 512])   # negated
    b_eff1 = din("b_eff1", [128, KC1])   # b_pse@W_p1[:E]+b_p1, col-chunked
    w_p1h = din("w_p1h", [H, 512])       # W_p1[E:]
    w_p2 = din("w_p2", [512, BN])
    b_p2 = din("b_p2", [128, DC])
    w_m1 = din("w_m1", [H + BN, MH])
    b_m1 = din("b_m1", [128, MH // 128])
    w_m2 = din("w_m2", [MH, H])
    b_m2 = din("b_m2", [H, 1])
    eg_d = din("eg", [t_steps * DC, 128, R])
    preds = nc.declare_dram_parameter("preds", [t_steps, 2, BLOC], FP, isOutput=True)

    with tile.TileContext(nc) as tc:
        with (
            tc.tile_pool(name="wpool", bufs=1) as wp,
            tc.tile_pool(name="x1pool", bufs=1) as x1p,
            tc.tile_pool(name="big", bufs=2) as bigp,
            tc.tile_pool(name="egpool", bufs=3) as egp,
            tc.tile_pool(name="state", bufs=2) as st,
            tc.tile_pool(name="psA", bufs=4, space="PSUM") as psA,
            tc.tile_pool(name="psB", bufs=3, space="PSUM") as psB,
        ):
            # ---- load weights once ----
            def wtile(dram, shape=None):
                t_ = wp.tile(shape or dram.shape, FP, name=dram.name + "_sb")
                nc.sync.dma_start(out=t_, in_=dram[:])
                return t_

            W_se = wtile(w_se)
            W_ih = wtile(w_ih)
            W_hh = wtile(w_hh)
            Bg = wtile(b_gates)
            W_hp = wtile(w_hp)
            Bhp = wtile(b_hp)
            W_e1 = wtile(w_eff1)
            nW_e1 = wtile(nw_eff1)
            Be1 = wtile(b_eff1)
            W_p1h = wtile(w_p1h)
            Bp2 = wtile(b_p2)
            Bm1 = wtile(b_m1)
            Bm2 = wtile(b_m2)
            W_p2 = wp.tile([128, KC1, BN], FP)      # [k-part, kc, d]
            for kc in range(KC1):
                nc.sync.dma_start(out=W_p2[:, kc, :], in_=w_p2[kc * 128:(kc + 1) * 128, :])
            W_m1 = wp.tile([128, 9, MH], FP)        # [k-part, kc(cat), m]
            for kc in range(9):
                nc.sync.dma_start(out=W_m1[:, kc, :], in_=w_m1[kc * 128:(kc + 1) * 128, :])
            W_m2 = wp.tile([128, DC, H], FP)        # [k-part, mc, h]
            for mc in range(DC):
                nc.sync.dma_start(out=W_m2[:, mc, :], in_=w_m2[mc * 128:(mc + 1) * 128, :])

            # ---- initial state ----
            h_T = st.tile([H, BLOC], FP, name="h_T")
            c_T = st.tile([H, BLOC], FP, name="c_T")
            pos_T = st.tile([2, BLOC], FP, name="pos_T")
            lr_T = st.tile([2, BLOC], FP, name="lr_T")
            nc.sync.dma_start(out=h_T, in_=h0[:])
            nc.sync.dma_start(out=c_T, in_=c0[:])
            nc.sync.dma_start(out=pos_T, in_=lp0[:])
            nc.sync.dma_start(out=lr_T, in_=lr0[:])

            for t in range(t_steps):
                # ===== A. LSTM cell =====
                x_ps = psB.tile([E, BLOC], FP, tag="psSmall", name="x_ps")
                nc.tensor.matmul(x_ps, W_se[:], lr_T[:])
                x_sb = st.tile([E, BLOC], FP, name="x_sb")
                nc.scalar.activation(out=x_sb, in_=x_ps, func=AF.Copy)

                g_ps = psB.tile([H, 4, BLOC], FP, tag="psSmall", name="g_ps")
                for gc in range(4):
                    nc.tensor.matmul(g_ps[:, gc, :], W_ih[:, gc * H:(gc + 1) * H],
                                     x_sb[:], start=True, stop=False)
                    nc.tensor.matmul(g_ps[:, gc, :], W_hh[:, gc * H:(gc + 1) * H],
                                     h_T[:], start=False, stop=True)
                sig_i = st.tile([H, BLOC], FP, name="sig_i")
                sig_f = st.tile([H, BLOC], FP, name="sig_f")
                tan_g = st.tile([H, BLOC], FP, name="tan_g")
                sig_o = st.tile([H, BLOC], FP, name="sig_o")
                nc.scalar.activation(out=sig_i, in_=g_ps[:, 0, :], func=AF.Sigmoid, bias=Bg[:, 0:1])
                nc.scalar.activation(out=sig_f, in_=g_ps[:, 1, :], func=AF.Sigmoid, bias=Bg[:, 1:2])
                nc.scalar.activation(out=tan_g, in_=g_ps[:, 2, :], func=AF.Tanh, bias=Bg[:, 2:3])
                nc.scalar.activation(out=sig_o, in_=g_ps[:, 3, :], func=AF.Sigmoid, bias=Bg[:, 3:4])
                t1 = st.tile([H, BLOC], FP, name="t1")
                c_new = st.tile([H, BLOC], FP, name="c_new")
                nc.vector.tensor_mul(t1, sig_f[:], c_T[:])
                nc.vector.tensor_mul(c_new, sig_i[:], tan_g[:])
                nc.vector.tensor_add(c_new, c_new[:], t1[:])
                tc_t = st.tile([H, BLOC], FP, name="tc_t")
                nc.scalar.activation(out=tc_t, in_=c_new[:], func=AF.Tanh)
                h_mid = st.tile([H, BLOC], FP, name="h_mid")
                nc.vector.tensor_mul(h_mid, sig_o[:], tc_t[:])
                c_T = c_new

                # rel_pos / curr_pos
                rp_ps = psB.tile([2, BLOC], FP, tag="psSmall", name="rp_ps")
                nc.tensor.matmul(rp_ps, W_hp[:], h_mid[:])
                relp = st.tile([2, BLOC], FP, name="relp")
                nc.vector.tensor_scalar_add(relp, rp_ps[:], Bhp[:, 0:1])
                pos_new = st.tile([2, BLOC], FP, name="pos_new")
                nc.vector.tensor_add(pos_new, relp[:], pos_T[:])
                nc.sync.dma_start(out=preds[t], in_=relp[:])
                pos_T = pos_new
                lr_T = relp

                # ===== B. pool matmul 1: x1_T[512, R] =====
                x1 = x1p.tile([128, KC1, R], FP, name="x1")
                for kc in range(KC1):
                    for nt in range(NT):
                        s_idx, i0 = nt // 2, (nt % 2) * 16
                        ps1 = psA.tile([128, 512], FP, tag="psMM", name="ps1")
                        hj = _bview(h_mid, s_idx * 32, [[0, 16], [1, 32]])
                        pj = _bview(pos_T, s_idx * 32, [[0, 16], [1, 32]])
                        pi = _bview(pos_T, s_idx * 32 + i0, [[1, 16], [0, 32]])
                        ksl = slice(kc * 128, (kc + 1) * 128)
                        nc.tensor.matmul(ps1, W_p1h[:, ksl], hj, start=True, stop=False)
                        nc.tensor.matmul(ps1, W_e1[:, ksl], pj, start=False, stop=False)
                        nc.tensor.matmul(ps1, nW_e1[:, ksl], pi, start=False, stop=True)
                        nc.scalar.activation(out=x1[:, kc, nt * 512:(nt + 1) * 512],
                                             in_=ps1, func=AF.Relu, bias=Be1[:, kc:kc + 1])

                # ===== C. pool matmul 2 + gumbel hard selection =====
                pooled = bigp.tile([128, DC, BLOC], FP, name="pooled")
                for mc in range(DC):
                    raw = bigp.tile([128, R], FP, name="raw")
                    for nt in range(NT):
                        ps2 = psA.tile([128, 512], FP, tag="psMM", name="ps2")
                        msl = slice(mc * 128, (mc + 1) * 128)
                        for kc in range(KC1):
                            nc.tensor.matmul(ps2, W_p2[:, kc, msl],
                                             x1[:, kc, nt * 512:(nt + 1) * 512],
                                             start=(kc == 0), stop=(kc == KC1 - 1))
                        nc.scalar.activation(out=raw[:, nt * 512:(nt + 1) * 512],
                                             in_=ps2, func=AF.Relu, bias=Bp2[:, mc:mc + 1])
                    eg_t = egp.tile([128, R], FP, name="eg_t")
                    nc.sync.dma_start(out=eg_t, in_=eg_d[t * DC + mc])
                    sc = bigp.tile([128, R], FP, name="sc")
                    eng_score.scalar_tensor_tensor(out=sc, in0=raw[:], scalar=LEPS,
                                                   in1=eg_t[:], op0=ALU.add, op1=ALU.mult)
                    mx = bigp.tile([128, BLOC], FP, name="mx")
                    v.tensor_reduce(out=mx, in_=sc.rearrange("p (g j) -> p g j", j=P),
                                    axis=AX.X, op=ALU.max)
                    mxb = _ap(mx[:], [[1, BLOC], [0, P]])
                    sc3 = sc.rearrange("p (g j) -> p g j", j=P)
                    eng_eq.tensor_tensor(sc3, sc3, mxb, ALU.is_equal)
                    eng_p1.tensor_mul(sc, sc[:], raw[:])
                    v.tensor_reduce(out=pooled[:, mc, :],
                                    in_=sc.rearrange("p (g j) -> p g j", j=P),
                                    axis=AX.X, op=ALU.add)

                # ===== D. MLP: h = relu(relu(cat@W_m1+b)@W_m2+b) =====
                m1_ps = psA.tile([128, MH // 2], FP, tag="psMM", name="m1_ps")
                m1b_ps = psA.tile([128, MH // 2], FP, tag="psMM", name="m1b_ps")
                h1 = bigp.tile([128, MH // 128, BLOC], FP, name="h1")
                for mc2 in range(MH // 128):
                    dst = (m1_ps if mc2 < 4 else m1b_ps)
                    osl = slice((mc2 % 4) * BLOC, (mc2 % 4 + 1) * BLOC)
                    msl = slice(mc2 * 128, (mc2 + 1) * 128)
                    nc.tensor.matmul(dst[:, osl], W_m1[:, 0, msl], h_mid[:],
                                     start=True, stop=False)
                    for j in range(DC):
                        nc.tensor.matmul(dst[:, osl], W_m1[:, 1 + j, msl],
                                         pooled[:, j, :], start=False, stop=(j == DC - 1))
                    nc.scalar.activation(out=h1[:, mc2, :], in_=dst[:, osl],
                                         func=AF.Relu, bias=Bm1[:, mc2:mc2 + 1])
                h2_ps = psB.tile([H, BLOC], FP, tag="psSmall", name="h2_ps")
                for mc2 in range(DC):
                    nc.tensor.matmul(h2_ps, W_m2[:, mc2, :], h1[:, mc2, :],
                                     start=(mc2 == 0), stop=(mc2 == DC - 1))
                h_new = st.tile([H, BLOC], FP, name="h_new")
                nc.scalar.activation(out=h_new, in_=h2_ps[:], func=AF.Relu, bias=Bm2[:, 0:1])
                h_T = h_new
    nc.compile()
    return nc


@functools.lru_cache(maxsize=1)
def _eg_host(t_steps=T):
    """Host-precomputed exp(gumbel) [t, S, P, P, BN] -> per-core chunked."""
    import jax
    import jax.numpy as jnp
    cpu = jax.devices("cpu")[0]
    out = np.empty((t_steps, S, P, P, BN), np.float32)
    with jax.default_device(cpu):
        key = jax.random.key(42)
        for t in range(t_steps):
            k = jax.random.fold_in(key, t)
            u = jax.random.uniform(k, (S, P, P, BN), dtype=jnp.float32)
            out[t] = np.asarray(1.0 / (GEPS - jnp.log(u + GEPS)))
    return out


def _prep_weights(inp):
    f = np.float32
    W_p1 = np.asarray(inp["W_p1"], f)
    W_pse = np.asarray(inp["W_pse"], f)
    w_eff1 = (W_pse @ W_p1[:E]).astype(f)
    b_eff1 = (np.asarray(inp["b_pse"], f) @ W_p1[:E] + np.asarray(inp["b_p1"], f)).astype(f)
    b_gates = (np.asarray(inp["b_se"], f) @ np.asarray(inp["W_ih"], f)
               + np.asarray(inp["b_ih"], f) + np.asarray(inp["b_hh"], f)).astype(f)
    return {
        "w_se": np.asarray(inp["W_se"], f),
        "w_ih": np.asarray(inp["W_ih"], f),
        "w_hh": np.asarray(inp["W_hh"], f),
        "b_gates": np.ascontiguousarray(b_gates.reshape(4, H).T),
        "w_hp": np.asarray(inp["W_hp"], f),
        "b_hp": np.asarray(inp["b_hp"], f).reshape(2, 1),
        "w_eff1": w_eff1,
        "nw_eff1": -w_eff1,
        "b_eff1": np.ascontiguousarray(b_eff1.reshape(KC1, 128).T),
        "w_p1h": np.ascontiguousarray(W_p1[E:]),
        "w_p2": np.asarray(inp["W_p2"], f),
        "b_p2": np.ascontiguousarray(np.asarray(inp["b_p2"], f).reshape(DC, 128).T),
        "w_m1": np.asarray(inp["W_m1"], f),
        "b_m1": np.ascontiguousarray(np.asarray(inp["b_m1"], f).reshape(MH // 128, 128).T),
        "w_m2": np.asarray(inp["W_m2"], f),
        "b_m2": np.asarray(inp["b_m2"], f).reshape(H, 1),
    }


_NC_CACHE = {}


def _get_nc(t_steps=T):
    if t_steps not in _NC_CACHE:
        _NC_CACHE[t_steps] = build_nc(t_steps)
    return _NC_CACHE[t_steps]


def make_in_maps(inputs, t_steps=T):
    """Per-core input dicts (also used by the test harness / simulator)."""
    w = _prep_weights(inputs)
    eg = _eg_host(t_steps)  # [t, S, P, P, BN]
    f = np.float32
    lp = np.asarray(inputs["last_pos"], f)
    lr = np.asarray(inputs["last_pos_rel"], f)
    h0 = np.asarray(inputs["h0"], f)
    c0 = np.asarray(inputs["c0"], f)
    in_maps = []
    for d in range(NCORES):
        rows = slice(d * BLOC, (d + 1) * BLOC)
        scenes = slice(d * SLOC, (d + 1) * SLOC)
        # eg slice -> [t, BN, sloc, P, P] -> [t*DC, 128, R]
        egd = np.ascontiguousarray(
            eg[:, scenes].transpose(0, 4, 1, 2, 3).reshape(t_steps * DC, 128, R))
        m = dict(w)
        m.update({
            "lr0": np.ascontiguousarray(lr[rows].T),
            "lp0": np.ascontiguousarray(lp[rows].T),
            "h0t": np.ascontiguousarray(h0[rows].T),
            "c0t": np.ascontiguousarray(c0[rows].T),
            "eg": egd,
        })
        in_maps.append(m)
    return in_maps


def kernel(**inputs):
    nc = _get_nc(T)
    in_maps = make_in_maps(inputs, T)
    res = run_bass_kernel_spmd(nc, in_maps, list(range(NCORES)))
    out = np.empty((T, B, 2), np.float32)
    for d in range(NCORES):
        pr = res.results[d]["preds"]  # [T, 2, BLOC]
        out[:, d * BLOC:(d + 1) * BLOC, :] = pr.transpose(0, 2, 1)
    return out
